# revision 2
# baseline (speedup 1.0000x reference)
"""Trainium2 Bass kernel for MAB (multihead attention block) — nn_MAB_48412871360901.

Data-parallel over batch: 16 batches -> 8 NeuronCores, 2 batches/core.
Per core, per batch (matmuls bf16 with fp32 PSUM accumulation):
  P1  Q,K loaded natural f32, PE-transposed (f32 in, f32 psum out), DVE copy
      casts to bf16 -> QT/KT  [dv, nq] layout
  P2  projections qT = Wq^T QT, kT = Wk^T KT (transposed layout), v = K Wv (natural)
  P3  per (head-pair, q-chunk): S^T = k^T.T q^T (row-packed 2 heads),
      exp on ACT (scale 1/sqrt(512) fused), softmax denominators Z via
      ones[128,32]-matmuls (rows 0..31 = Z_h0, rows 32..63 = Z_h1, all finite),
      1/Z = exp(-ln Z) on ACT (rows 0..32), broadcast via mkJ-matmul,
      PV U^T = v^T expS^T (col-packed 2 heads),
      X^T = U^T * (1/Z)bcast + qT   (residual uses post-projection q)
  P4  LN0 in transposed layout: stats via ones-matmuls over partitions,
      rstd = exp(-0.5 ln(var+eps)) on ACT, normalize with PE-broadcast tiles
  P5  M = Xn Wo (natural out) + identity-fold transpose of Xn; relu; residual;
      LN1 natural (bn_stats); DMA out fp32.
PSUM (8 banks): wide 2x[128,1024]f32 (scores/b2/m), half 2x[128,512]f32
(tp/pp/st), zz 1x (z then rz), pv 1x (u accum).
"""

import sys
from contextlib import ExitStack
import numpy as np
import ml_dtypes

for _p in ("/opt/trn_rl_repo", "/root/.axon_site/_ro/trn_rl_repo"):
    if _p not in sys.path:
        sys.path.insert(0, _p)

import concourse.bacc as bacc
import concourse.mybir as mybir
import concourse.tile as tile
from concourse.bass_utils import run_bass_kernel_spmd

BF16 = mybir.dt.bfloat16
F32 = mybir.dt.float32
NBF = ml_dtypes.bfloat16
AF = mybir.ActivationFunctionType
OP = mybir.AluOpType

B, NQ, NK = 16, 1024, 1024
D = 512
H = 8
N_CORES = 8
BL = B // N_CORES          # batches per core
EPS = 1e-5
SCALE = 1.0 / np.sqrt(512.0)

_cache = {}


class _Ctx:
    pass


def _setup_consts(nc, cx, cst, flags):
    (bq_nz, bk_nz, bv_nz, bo_nz, ln0_aff, ln1_aff) = flags

    def din(name, shape, dt=BF16):
        return nc.dram_tensor(name, list(shape), dt, kind="ExternalInput").ap()

    def ldc(name, dshape, shape, rearr=None):
        d = din(name, dshape)
        t = cst.tile(list(shape), BF16, tag=name)
        nc.sync.dma_start(out=t, in_=d if rearr is None else d.rearrange(rearr, p=128))
        return t

    def ldf(name, shape):
        d = din(name, shape, F32)
        t = cst.tile(list(shape), F32, tag=name)
        nc.sync.dma_start(out=t, in_=d)
        return t

    cx.w_q = ldc("Wqb", (D, D), (128, 4, D), "(kt p) c -> p kt c")
    cx.w_k = ldc("Wkb", (D, D), (128, 4, D), "(kt p) c -> p kt c")
    cx.w_v = ldc("Wvb", (D, D), (128, 4, D), "(kt p) c -> p kt c")
    cx.w_o = ldc("Wob", (D, D), (128, 4, D), "(kt p) c -> p kt c")
    cx.i512 = ldc("I512b", (D, D), (128, 4, D), "(kt p) c -> p kt c")
    cx.i128f = ldf("I128f", (128, 128))
    cx.onesc = ldc("onesc", (128, 1), (128, 1))
    cx.onesr = ldc("onesr", (1, 128), (1, 128))
    cx.ones32 = ldc("ones32", (128, 32), (128, 32))
    cx.mkJ = ldc("mkJ", (33, 128), (33, 128))
    cx.epsP = cst.tile([128, 1], F32, tag="epsP"); nc.vector.memset(cx.epsP, EPS)
    cx.eps1 = cst.tile([1, 1], F32, tag="eps1"); nc.vector.memset(cx.eps1, EPS)
    cx.bq4 = ldf("bq4", (128, 4)) if bq_nz else None
    cx.bk4 = ldf("bk4", (128, 4)) if bk_nz else None
    cx.bvb = ldf("bvb", (128, D)) if bv_nz else None
    cx.bob = ldf("bob", (128, D)) if bo_nz else None
    cx.g04 = ldf("g04", (128, 4)) if ln0_aff else None
    cx.b04 = ldf("b04", (128, 4)) if ln0_aff else None
    cx.g1b = ldf("g1b", (128, D)) if ln1_aff else None
    cx.b1b = ldf("b1b", (128, D)) if ln1_aff else None


def _p1_transpose(nc, cx, rb, src_dram, tag):
    dst = cx.p_qkt.tile([128, 4, NQ], BF16, tag=tag)
    for i in range(8):
        nat = cx.p_nat.tile([128, D], F32)
        nc.sync.dma_start(out=nat, in_=src_dram[rb + 128 * i: rb + 128 * (i + 1), :])
        tp = cx.ps_half.tile([128, D], F32, tag="h")
        for j in range(4):
            nc.tensor.transpose(tp[:, 128 * j:128 * (j + 1)],
                                nat[:, 128 * j:128 * (j + 1)], cx.i128f)
        nc.vector.tensor_copy(
            out=dst[:, :, 128 * i:128 * (i + 1)],
            in_=tp.rearrange("p (j c) -> p j c", j=4))
    return dst


def _p2_proj(nc, cx, QT, KT):
    qT = cx.p_proj.tile([128, 4, NQ], BF16, tag="qT")
    kT = cx.p_proj.tile([128, 4, NQ], BF16, tag="kT")
    vT = cx.p_proj.tile([128, 8, D], BF16, tag="vT")

    for dst, w, srcT, bias in ((qT, cx.w_q, QT, cx.bq4), (kT, cx.w_k, KT, cx.bk4)):
        for dvt in range(4):
            for qc in range(2):
                pp = cx.ps_half.tile([128, D], F32, tag="h")
                for kt in range(4):
                    nc.tensor.matmul(
                        pp, lhsT=w[:, kt, 128 * dvt:128 * (dvt + 1)],
                        rhs=srcT[:, kt, 512 * qc:512 * (qc + 1)],
                        start=(kt == 0), stop=(kt == 3))
                o = dst[:, dvt, 512 * qc:512 * (qc + 1)]
                if bias is not None:
                    nc.vector.tensor_scalar_add(out=o, in0=pp, scalar1=bias[:, dvt:dvt + 1])
                else:
                    nc.vector.tensor_copy(out=o, in_=pp)
    for nkt in range(8):
        pp = cx.ps_half.tile([128, D], F32, tag="h")
        for kt in range(4):
            nc.tensor.matmul(pp, lhsT=KT[:, kt, 128 * nkt:128 * (nkt + 1)],
                             rhs=cx.w_v[:, kt, :], start=(kt == 0), stop=(kt == 3))
        if cx.bvb is not None:
            nc.vector.scalar_tensor_tensor(out=vT[:, nkt, :], in0=pp, scalar=0.0,
                                           in1=cx.bvb, op0=OP.add, op1=OP.add)
        else:
            nc.vector.tensor_copy(out=vT[:, nkt, :], in_=pp)
    return qT, kT, vT


def _p3_attn_unit(nc, cx, qT, kT, vT, XT, SQ, hp, qc):
    ps_u = cx.ps_pv.tile([128, D], F32, tag="u")
    ps_z = cx.ps_zz.tile([128, D], F32, tag="z")
    for kt in range(8):
        ps_s = cx.ps_wide.tile([128, 2 * D], F32, tag="w")
        nc.tensor.matmul(
            ps_s[:, 0:D],
            lhsT=kT[0:64, hp, 128 * kt:128 * (kt + 1)],
            rhs=qT[0:64, hp, 512 * qc:512 * (qc + 1)],
            start=True, stop=True, tile_position=(0, 0))
        nc.tensor.matmul(
            ps_s[:, D:2 * D],
            lhsT=kT[64:128, hp, 128 * kt:128 * (kt + 1)],
            rhs=qT[64:128, hp, 512 * qc:512 * (qc + 1)],
            start=True, stop=True, tile_position=(64, 0))
        ex = cx.p_ex.tile([128, 2 * D], BF16)
        nc.scalar.activation(out=ex, in_=ps_s, func=AF.Exp, scale=SCALE)
        nc.tensor.matmul(ps_z[0:32, :], lhsT=cx.ones32, rhs=ex[:, 0:D],
                         start=(kt == 0), stop=(kt == 7), tile_position=(0, 0))
        nc.tensor.matmul(ps_z[32:64, :], lhsT=cx.ones32, rhs=ex[:, D:2 * D],
                         start=(kt == 0), stop=(kt == 7), tile_position=(0, 32))
        nc.tensor.matmul(ps_u[0:64, :],
                         lhsT=vT[:, kt, 128 * hp:128 * hp + 64],
                         rhs=ex[:, 0:D],
                         start=(kt == 0), stop=(kt == 7), tile_position=(0, 0))
        nc.tensor.matmul(ps_u[64:128, :],
                         lhsT=vT[:, kt, 128 * hp + 64:128 * (hp + 1)],
                         rhs=ex[:, D:2 * D],
                         start=(kt == 0), stop=(kt == 7), tile_position=(0, 64))
    # 1/Z = exp(-ln Z) on ACT; rows 0..31 hold Z_h0, row 32 holds Z_h1 (finite)
    lnz = cx.p_sml.tile([33, D], F32, tag="lnz")
    nc.scalar.activation(out=lnz, in_=ps_z[0:33, :], func=AF.Ln, scale=1.0)
    rz33 = cx.p_sml.tile([33, D], BF16, tag="rz33")
    nc.scalar.activation(out=rz33, in_=lnz, func=AF.Exp, scale=-1.0)
    ps_rz = cx.ps_zz.tile([128, D], F32, tag="z")
    nc.tensor.matmul(ps_rz, lhsT=cx.mkJ, rhs=rz33, start=True, stop=True)
    u_bf = cx.p_ub.tile([128, D], BF16)
    nc.vector.tensor_copy(out=u_bf, in_=ps_u)
    t1 = cx.p_t1.tile([128, D], BF16, tag="t")
    nc.vector.tensor_tensor(out=t1, in0=u_bf, in1=ps_rz, op=OP.mult)
    xs = XT[:, hp, 512 * qc:512 * (qc + 1)]
    nc.vector.tensor_tensor(out=xs, in0=t1,
                            in1=qT[:, hp, 512 * qc:512 * (qc + 1)], op=OP.add)
    nc.vector.tensor_tensor(out=SQ[:, hp, 512 * qc:512 * (qc + 1)],
                            in0=xs, in1=xs, op=OP.mult)


def _p4_ln0(nc, cx, XT, SQ, ln0_aff):
    XnT = cx.p_xnt.tile([128, 4, NQ], BF16)
    for qc in range(2):
        ps_st = cx.ps_half.tile([128, D], F32, tag="h")
        for dvt in range(4):
            nc.tensor.matmul(ps_st[0:1, :], lhsT=cx.onesc,
                             rhs=XT[:, dvt, 512 * qc:512 * (qc + 1)],
                             start=(dvt == 0), stop=(dvt == 3), tile_position=(0, 0))
            nc.tensor.matmul(ps_st[32:33, :], lhsT=cx.onesc,
                             rhs=SQ[:, dvt, 512 * qc:512 * (qc + 1)],
                             start=(dvt == 0), stop=(dvt == 3), tile_position=(0, 32))
        mu = cx.p_sml.tile([1, D], F32, tag="mu")
        nc.vector.tensor_scalar_mul(out=mu, in0=ps_st[0:1, :], scalar1=1.0 / D)
        mu2 = cx.p_sml.tile([1, D], F32, tag="mu2")
        nc.vector.tensor_tensor(out=mu2, in0=mu, in1=mu, op=OP.mult)
        var = cx.p_sml.tile([1, D], F32, tag="var")
        nc.vector.scalar_tensor_tensor(out=var, in0=ps_st[32:33, :],
                                       scalar=1.0 / D, in1=mu2,
                                       op0=OP.mult, op1=OP.subtract)
        lnv = cx.p_sml.tile([1, D], F32, tag="lnv")
        nc.scalar.activation(out=lnv, in_=var, func=AF.Ln, bias=cx.eps1, scale=1.0)
        rstd = cx.p_sml.tile([1, D], BF16, tag="rstd")
        nc.scalar.activation(out=rstd, in_=lnv, func=AF.Exp, scale=-0.5)
        nmr = cx.p_sml.tile([1, D], BF16, tag="nmr")
        nc.vector.scalar_tensor_tensor(out=nmr, in0=mu, scalar=-1.0, in1=rstd,
                                       op0=OP.mult, op1=OP.mult)
        ps_b2 = cx.ps_wide.tile([128, 2 * D], F32, tag="w")
        nc.tensor.matmul(ps_b2[:, 0:D], lhsT=cx.onesr, rhs=rstd, start=True, stop=True)
        nc.tensor.matmul(ps_b2[:, D:2 * D], lhsT=cx.onesr, rhs=nmr, start=True, stop=True)
        for dvt in range(4):
            t2 = cx.p_t1.tile([128, D], BF16, tag="t")
            nc.vector.tensor_tensor(out=t2, in0=XT[:, dvt, 512 * qc:512 * (qc + 1)],
                                    in1=ps_b2[:, 0:D], op=OP.mult)
            xn = XnT[:, dvt, 512 * qc:512 * (qc + 1)]
            nc.vector.tensor_tensor(out=xn, in0=t2, in1=ps_b2[:, D:2 * D], op=OP.add)
            if ln0_aff:
                nc.vector.tensor_scalar(out=xn, in0=xn,
                                        scalar1=cx.g04[:, dvt:dvt + 1],
                                        scalar2=cx.b04[:, dvt:dvt + 1],
                                        op0=OP.mult, op1=OP.add)
    return XnT


def _p5_out(nc, cx, XnT, dOut, rb, ln1_aff):
    xpre_l, mv_l = [], []
    vars8 = cx.p_sml.tile([128, 8], F32, tag="vars8")
    for nqt in range(8):
        ps_m = cx.ps_wide.tile([128, 2 * D], F32, tag="w")
        for dvt in range(4):
            lb = XnT[:, dvt, 128 * nqt:128 * (nqt + 1)]
            nc.tensor.matmul(ps_m[:, 0:D], lhsT=lb, rhs=cx.w_o[:, dvt, :],
                             start=(dvt == 0), stop=(dvt == 3))
            nc.tensor.matmul(ps_m[:, D:2 * D], lhsT=lb, rhs=cx.i512[:, dvt, :],
                             start=(dvt == 0), stop=(dvt == 3))
        rl = cx.p_t1.tile([128, D], BF16, tag="t")
        if cx.bob is not None:
            tb = cx.p_t1.tile([128, D], BF16, tag="t")
            nc.vector.tensor_tensor(out=tb, in0=cx.bob, in1=ps_m[:, 0:D], op=OP.add)
            nc.vector.tensor_scalar_max(out=rl, in0=tb, scalar1=0.0)
        else:
            nc.vector.tensor_scalar_max(out=rl, in0=ps_m[:, 0:D], scalar1=0.0)
        xpre = cx.p_xp.tile([128, D], F32)
        nc.vector.tensor_tensor(out=xpre, in0=rl, in1=ps_m[:, D:2 * D], op=OP.add)
        bst = cx.p_mv.tile([128, 6], F32, tag="bst")
        nc.vector.bn_stats(out=bst, in_=xpre)
        mv = cx.p_mv.tile([128, 2], F32, tag="mv")
        nc.vector.bn_aggr(out=mv, in_=bst)
        nc.vector.tensor_copy(out=vars8[:, nqt:nqt + 1], in_=mv[:, 1:2])
        xpre_l.append(xpre); mv_l.append(mv)
    lnv8 = cx.p_sml.tile([128, 8], F32, tag="lnv8")
    nc.scalar.activation(out=lnv8, in_=vars8, func=AF.Ln, bias=cx.epsP, scale=1.0)
    rstd8 = cx.p_sml.tile([128, 8], F32, tag="rstd8")
    nc.scalar.activation(out=rstd8, in_=lnv8, func=AF.Exp, scale=-0.5)
    for nqt in range(8):
        ot = cx.p_out.tile([128, D], F32)
        nc.vector.tensor_scalar(out=ot, in0=xpre_l[nqt],
                                scalar1=mv_l[nqt][:, 0:1],
                                scalar2=rstd8[:, nqt:nqt + 1],
                                op0=OP.subtract, op1=OP.mult)
        if ln1_aff:
            nc.vector.tensor_tensor(out=ot, in0=ot, in1=cx.g1b, op=OP.mult)
            nc.vector.tensor_tensor(out=ot, in0=ot, in1=cx.b1b, op=OP.add)
        nc.sync.dma_start(out=dOut[rb + 128 * nqt: rb + 128 * (nqt + 1), :], in_=ot)


def _build(flags, repeat=1):
    (bq_nz, bk_nz, bv_nz, bo_nz, ln0_aff, ln1_aff) = flags
    nc = bacc.Bacc("TRN2", target_bir_lowering=False, debug=False,
                   num_devices=N_CORES)

    dQ = nc.dram_tensor("Qs", [BL * NQ, D], F32, kind="ExternalInput").ap()
    dK = nc.dram_tensor("Ks", [BL * NK, D], F32, kind="ExternalInput").ap()
    dOut = nc.dram_tensor("OUT", [BL * NQ, D], F32, kind="ExternalOutput").ap()

    cx = _Ctx()
    with ExitStack() as es:
        tc = es.enter_context(tile.TileContext(nc))
        ec = es.enter_context
        cst = ec(tc.tile_pool(name="cst", bufs=1))
        cx.p_qkt = ec(tc.tile_pool(name="qkt", bufs=1))
        cx.p_proj = ec(tc.tile_pool(name="proj", bufs=2))
        cx.p_xt = ec(tc.tile_pool(name="xt", bufs=1))
        cx.p_xnt = ec(tc.tile_pool(name="xnt", bufs=2))
        cx.p_nat = ec(tc.tile_pool(name="nat", bufs=2))
        cx.p_ex = ec(tc.tile_pool(name="ex", bufs=4))
        cx.p_ub = ec(tc.tile_pool(name="ub", bufs=2))
        cx.p_t1 = ec(tc.tile_pool(name="t1", bufs=3))
        cx.p_xp = ec(tc.tile_pool(name="xp", bufs=9))
        cx.p_out = ec(tc.tile_pool(name="outp", bufs=2))
        cx.p_sml = ec(tc.tile_pool(name="sml", bufs=2))
        cx.p_mv = ec(tc.tile_pool(name="mv", bufs=10))
        cx.ps_wide = ec(tc.tile_pool(name="wide", bufs=2, space="PSUM"))
        cx.ps_half = ec(tc.tile_pool(name="half", bufs=2, space="PSUM"))
        cx.ps_zz = ec(tc.tile_pool(name="zz", bufs=1, space="PSUM"))
        cx.ps_pv = ec(tc.tile_pool(name="pv", bufs=1, space="PSUM"))
        _setup_consts(nc, cx, cst, flags)

        def body():
            for b in range(BL):
                rb = b * NQ
                QT = _p1_transpose(nc, cx, rb, dQ, "QT")
                KT = _p1_transpose(nc, cx, rb, dK, "KT")
                qT, kT, vT = _p2_proj(nc, cx, QT, KT)
                XT = cx.p_xt.tile([128, 4, NQ], BF16, tag="XT")
                SQ = cx.p_xt.tile([128, 4, NQ], BF16, tag="SQ")
                for hp in range(4):
                    for qc in range(2):
                        _p3_attn_unit(nc, cx, qT, kT, vT, XT, SQ, hp, qc)
                XnT = _p4_ln0(nc, cx, XT, SQ, ln0_aff)
                _p5_out(nc, cx, XnT, dOut, rb, ln1_aff)

        if repeat == 1:
            body()
        else:
            with tc.For_i(0, repeat, 1):
                body()

    nc.compile()
    return nc


def _consts(Wq, Wk, Wv, Wo, flags, bq, bk, bv, bo, g0, b0, g1, b1):
    (bq_nz, bk_nz, bv_nz, bo_nz, ln0_aff, ln1_aff) = flags
    c = {
        "Wqb": np.ascontiguousarray(np.asarray(Wq).astype(NBF)),
        "Wkb": np.ascontiguousarray(np.asarray(Wk).astype(NBF)),
        "Wvb": np.ascontiguousarray(np.asarray(Wv).astype(NBF)),
        "Wob": np.ascontiguousarray(np.asarray(Wo).astype(NBF)),
        "I512b": np.eye(D, dtype=NBF),
        "I128f": np.eye(128, dtype=np.float32),
        "onesc": np.ones((128, 1), NBF),
        "onesr": np.ones((1, 128), NBF),
        "ones32": np.ones((128, 32), NBF),
    }
    mkJ = np.zeros((33, 128), NBF)
    mkJ[0, :64] = 1
    mkJ[32, 64:] = 1
    c["mkJ"] = mkJ
    if bq_nz: c["bq4"] = np.ascontiguousarray(np.asarray(bq).reshape(4, 128).T.astype(np.float32))
    if bk_nz: c["bk4"] = np.ascontiguousarray(np.asarray(bk).reshape(4, 128).T.astype(np.float32))
    if bv_nz: c["bvb"] = np.ascontiguousarray(np.broadcast_to(np.asarray(bv, np.float32), (128, D)))
    if bo_nz: c["bob"] = np.ascontiguousarray(np.broadcast_to(np.asarray(bo, np.float32), (128, D)))
    if ln0_aff:
        c["g04"] = np.ascontiguousarray(np.asarray(g0).reshape(4, 128).T.astype(np.float32))
        c["b04"] = np.ascontiguousarray(np.asarray(b0).reshape(4, 128).T.astype(np.float32))
    if ln1_aff:
        c["g1b"] = np.ascontiguousarray(np.broadcast_to(np.asarray(g1, np.float32), (128, D)))
        c["b1b"] = np.ascontiguousarray(np.broadcast_to(np.asarray(b1, np.float32), (128, D)))
    return c


def make_in_maps(Q, K, Wq, bq, Wk, bk, Wv, bv, Wo, bo, g0, b0, g1, b1, flags):
    consts = _consts(Wq, Wk, Wv, Wo, flags, bq, bk, bv, bo, g0, b0, g1, b1)
    in_maps = []
    for ci in range(N_CORES):
        m = dict(consts)
        m["Qs"] = np.ascontiguousarray(
            np.asarray(Q)[ci * BL:(ci + 1) * BL].reshape(BL * NQ, D).astype(np.float32))
        m["Ks"] = np.ascontiguousarray(
            np.asarray(K)[ci * BL:(ci + 1) * BL].reshape(BL * NK, D).astype(np.float32))
        in_maps.append(m)
    return in_maps


def get_flags(bq, bk, bv, bo, g0, b0, g1, b1):
    return (bool(np.any(np.asarray(bq))), bool(np.any(np.asarray(bk))),
            bool(np.any(np.asarray(bv))), bool(np.any(np.asarray(bo))),
            bool(np.any(np.asarray(g0) != 1) or np.any(np.asarray(b0))),
            bool(np.any(np.asarray(g1) != 1) or np.any(np.asarray(b1))))


def get_program(flags, repeat=1):
    key = (flags, repeat)
    if key not in _cache:
        _cache[key] = _build(flags, repeat)
    return _cache[key]


def kernel(Q, K, Wq, bq, Wk, bk, Wv, bv, Wo, bo, g0, b0, g1, b1):
    flags = get_flags(bq, bk, bv, bo, g0, b0, g1, b1)
    nc = get_program(flags, repeat=1)
    in_maps = make_in_maps(Q, K, Wq, bq, Wk, bk, Wv, bv, Wo, bo, g0, b0, g1, b1, flags)
    res = run_bass_kernel_spmd(nc, in_maps, list(range(N_CORES)))
    out = np.empty((B, NQ, D), np.float32)
    for ci in range(N_CORES):
        out[ci * BL:(ci + 1) * BL] = res.results[ci]["OUT"].reshape(BL, NQ, D)
    return out


# revision 5
# speedup vs baseline: 1.3404x; 1.3404x over previous
"""Trainium2 Bass kernel for MAB (multihead attention block) — nn_MAB_48412871360901.

Data-parallel over batch: 16 batches -> 8 NeuronCores, 2 batches/core.
Per core, per batch (matmuls bf16 with fp32 PSUM accumulation):
  P1  Q,K loaded natural f32, PE-transposed (f32 in, f32 psum out), DVE copy
      casts to bf16 -> QT/KT  [dv, nq] layout
  P2  projections qT = Wq^T QT, kT = Wk^T KT (transposed layout), v = K Wv (natural)
  P3  per (head-pair, q-chunk): S^T = k^T.T q^T (row-packed 2 heads),
      exp on ACT (scale 1/sqrt(512) fused), softmax denominators Z via
      ones[128,32]-matmuls (rows 0..31 = Z_h0, rows 32..63 = Z_h1, all finite),
      1/Z = exp(-ln Z) on ACT (rows 0..32), broadcast via mkJ-matmul,
      PV U^T = v^T expS^T (col-packed 2 heads),
      X^T = U^T * (1/Z)bcast + qT   (residual uses post-projection q)
  P4  LN0 in transposed layout: stats via ones-matmuls over partitions,
      rstd = exp(-0.5 ln(var+eps)) on ACT, normalize with PE-broadcast tiles
  P5  M = Xn Wo (natural out) + identity-fold transpose of Xn; relu; residual;
      LN1 natural (bn_stats); DMA out fp32.
PSUM (8 banks): wide 2x[128,1024]f32 (scores/b2/m), half 2x[128,512]f32
(tp/pp/st), zz 1x (z then rz), pv 1x (u accum).
"""

import sys
from contextlib import ExitStack
import numpy as np
import ml_dtypes

for _p in ("/opt/trn_rl_repo", "/root/.axon_site/_ro/trn_rl_repo"):
    if _p not in sys.path:
        sys.path.insert(0, _p)

import concourse.bacc as bacc
import concourse.mybir as mybir
import concourse.tile as tile
from concourse.bass_utils import run_bass_kernel_spmd

BF16 = mybir.dt.bfloat16
F32 = mybir.dt.float32
NBF = ml_dtypes.bfloat16
AF = mybir.ActivationFunctionType
OP = mybir.AluOpType

B, NQ, NK = 16, 1024, 1024
D = 512
H = 8
N_CORES = 8
BL = B // N_CORES          # batches per core
EPS = 1e-5
SCALE = 1.0 / np.sqrt(512.0)

_cache = {}

# Pin Exp/Ln to the one table set that holds both, so the act-table-load
# pass keeps natural_log_exp_and_others resident instead of thrashing
# between exp_and_others and natural_log (~1.3us per switch, ~45 switches).
# Indices into act_info.json stay valid: we only filter membership, we do
# not reorder or drop sets.
_orig_gat = None


def _patched_gat(arch):
    t = dict(_orig_gat(arch))
    target = "natural_log_exp_and_others"
    if target in t and AF.Exp in t[target] and AF.Ln in t[target]:
        for name in t:
            if name != target:
                t[name] = t[name] - {AF.Exp, AF.Ln}
    return t


def _pin_act_tables():
    global _orig_gat
    if _orig_gat is None:
        _orig_gat = bacc.get_activation_tables
        bacc.get_activation_tables = _patched_gat


class _Ctx:
    pass


def _setup_consts(nc, cx, cst, flags):
    (bq_nz, bk_nz, bv_nz, bo_nz, ln0_aff, ln1_aff) = flags

    def din(name, shape, dt=BF16):
        return nc.dram_tensor(name, list(shape), dt, kind="ExternalInput").ap()

    def ldc(name, dshape, shape, rearr=None):
        d = din(name, dshape)
        t = cst.tile(list(shape), BF16, tag=name)
        nc.sync.dma_start(out=t, in_=d if rearr is None else d.rearrange(rearr, p=128))
        return t

    def ldf(name, shape):
        d = din(name, shape, F32)
        t = cst.tile(list(shape), F32, tag=name)
        nc.sync.dma_start(out=t, in_=d)
        return t

    cx.w_q = ldc("Wqb", (D, D), (128, 4, D), "(kt p) c -> p kt c")
    cx.w_k = ldc("Wkb", (D, D), (128, 4, D), "(kt p) c -> p kt c")
    cx.w_v = ldc("Wvb", (D, D), (128, 4, D), "(kt p) c -> p kt c")
    cx.w_o = ldc("Wob", (D, D), (128, 4, D), "(kt p) c -> p kt c")
    cx.i512 = ldc("I512b", (D, D), (128, 4, D), "(kt p) c -> p kt c")
    cx.i128f = ldf("I128f", (128, 128))
    cx.onesc = ldc("onesc", (128, 1), (128, 1))
    cx.onesr = ldc("onesr", (1, 128), (1, 128))
    cx.ones32 = ldc("ones32", (128, 32), (128, 32))
    cx.mkJ = ldc("mkJ", (33, 128), (33, 128))
    cx.epsP = cst.tile([128, 1], F32, tag="epsP"); nc.vector.memset(cx.epsP, EPS)
    cx.eps1 = cst.tile([1, 1], F32, tag="eps1"); nc.vector.memset(cx.eps1, EPS)
    cx.bq4 = ldf("bq4", (128, 4)) if bq_nz else None
    cx.bk4 = ldf("bk4", (128, 4)) if bk_nz else None
    cx.bvb = ldf("bvb", (128, D)) if bv_nz else None
    cx.bob = ldf("bob", (128, D)) if bo_nz else None
    cx.g04 = ldf("g04", (128, 4)) if ln0_aff else None
    cx.b04 = ldf("b04", (128, 4)) if ln0_aff else None
    cx.g1b = ldf("g1b", (128, D)) if ln1_aff else None
    cx.b1b = ldf("b1b", (128, D)) if ln1_aff else None


def _p1_transpose(nc, cx, rb, src_dram, tag):
    dst = cx.p_qkt.tile([128, 4, NQ], BF16, tag=tag)
    for i in range(8):
        nat = cx.p_nat.tile([128, D], F32)
        nc.sync.dma_start(out=nat, in_=src_dram[rb + 128 * i: rb + 128 * (i + 1), :])
        tp = cx.ps_half.tile([128, D], F32, tag="h")
        for j in range(4):
            nc.tensor.transpose(tp[:, 128 * j:128 * (j + 1)],
                                nat[:, 128 * j:128 * (j + 1)], cx.i128f)
        nc.vector.tensor_copy(
            out=dst[:, :, 128 * i:128 * (i + 1)],
            in_=tp.rearrange("p (j c) -> p j c", j=4))
    return dst


def _p2_proj(nc, cx, QT, KT):
    qT = cx.p_proj.tile([128, 4, NQ], BF16, tag="qT")
    kT = cx.p_proj.tile([128, 4, NQ], BF16, tag="kT")
    vT = cx.p_proj.tile([128, 8, D], BF16, tag="vT")

    for dst, w, srcT, bias in ((qT, cx.w_q, QT, cx.bq4), (kT, cx.w_k, KT, cx.bk4)):
        for dvt in range(4):
            for qc in range(2):
                pp = cx.ps_half.tile([128, D], F32, tag="h")
                for kt in range(4):
                    nc.tensor.matmul(
                        pp, lhsT=w[:, kt, 128 * dvt:128 * (dvt + 1)],
                        rhs=srcT[:, kt, 512 * qc:512 * (qc + 1)],
                        start=(kt == 0), stop=(kt == 3))
                o = dst[:, dvt, 512 * qc:512 * (qc + 1)]
                if bias is not None:
                    nc.vector.tensor_scalar_add(out=o, in0=pp, scalar1=bias[:, dvt:dvt + 1])
                else:
                    nc.vector.tensor_copy(out=o, in_=pp)
    for nkt in range(8):
        pp = cx.ps_half.tile([128, D], F32, tag="h")
        for kt in range(4):
            nc.tensor.matmul(pp, lhsT=KT[:, kt, 128 * nkt:128 * (nkt + 1)],
                             rhs=cx.w_v[:, kt, :], start=(kt == 0), stop=(kt == 3))
        if cx.bvb is not None:
            nc.vector.scalar_tensor_tensor(out=vT[:, nkt, :], in0=pp, scalar=0.0,
                                           in1=cx.bvb, op0=OP.add, op1=OP.add)
        else:
            nc.vector.tensor_copy(out=vT[:, nkt, :], in_=pp)
    return qT, kT, vT


def _p3_attn_unit(nc, cx, qT, kT, vT, XT, SQ, hp, qc):
    ps_u = cx.ps_pv.tile([128, D], F32, tag="u")
    ps_z = cx.ps_zz.tile([128, D], F32, tag="z")
    for kt in range(8):
        ps_s = cx.ps_wide.tile([128, 2 * D], F32, tag="w")
        nc.tensor.matmul(
            ps_s[:, 0:D],
            lhsT=kT[0:64, hp, 128 * kt:128 * (kt + 1)],
            rhs=qT[0:64, hp, 512 * qc:512 * (qc + 1)],
            start=True, stop=True, tile_position=(0, 0))
        nc.tensor.matmul(
            ps_s[:, D:2 * D],
            lhsT=kT[64:128, hp, 128 * kt:128 * (kt + 1)],
            rhs=qT[64:128, hp, 512 * qc:512 * (qc + 1)],
            start=True, stop=True, tile_position=(64, 0))
        ex = cx.p_ex.tile([128, 2 * D], BF16)
        nc.scalar.activation(out=ex, in_=ps_s, func=AF.Exp, scale=SCALE)
        nc.tensor.matmul(ps_z[0:32, :], lhsT=cx.ones32, rhs=ex[:, 0:D],
                         start=(kt == 0), stop=(kt == 7), tile_position=(0, 0))
        nc.tensor.matmul(ps_z[32:64, :], lhsT=cx.ones32, rhs=ex[:, D:2 * D],
                         start=(kt == 0), stop=(kt == 7), tile_position=(0, 32))
        nc.tensor.matmul(ps_u[0:64, :],
                         lhsT=vT[:, kt, 128 * hp:128 * hp + 64],
                         rhs=ex[:, 0:D],
                         start=(kt == 0), stop=(kt == 7), tile_position=(0, 0))
        nc.tensor.matmul(ps_u[64:128, :],
                         lhsT=vT[:, kt, 128 * hp + 64:128 * (hp + 1)],
                         rhs=ex[:, D:2 * D],
                         start=(kt == 0), stop=(kt == 7), tile_position=(0, 64))
    # 1/Z = exp(-ln Z) on ACT; rows 0..31 hold Z_h0, row 32 holds Z_h1 (finite)
    lnz = cx.p_sml.tile([33, D], F32, tag="lnz")
    nc.scalar.activation(out=lnz, in_=ps_z[0:33, :], func=AF.Ln, scale=1.0)
    rz33 = cx.p_sml.tile([33, D], BF16, tag="rz33")
    nc.scalar.activation(out=rz33, in_=lnz, func=AF.Exp, scale=-1.0)
    ps_rz = cx.ps_half.tile([128, D], F32, tag="h")
    nc.tensor.matmul(ps_rz, lhsT=cx.mkJ, rhs=rz33, start=True, stop=True)
    u_bf = cx.p_ub.tile([128, D], BF16)
    nc.vector.tensor_copy(out=u_bf, in_=ps_u)
    t1 = cx.p_t1.tile([128, D], BF16, tag="t")
    nc.vector.tensor_tensor(out=t1, in0=u_bf, in1=ps_rz, op=OP.mult)
    xs = XT[:, hp, 512 * qc:512 * (qc + 1)]
    nc.vector.tensor_tensor(out=xs, in0=t1,
                            in1=qT[:, hp, 512 * qc:512 * (qc + 1)], op=OP.add)
    nc.vector.tensor_tensor(out=SQ[:, hp, 512 * qc:512 * (qc + 1)],
                            in0=xs, in1=xs, op=OP.mult)


def _p4_ln0(nc, cx, XT, SQ, ln0_aff):
    XnT = cx.p_xnt.tile([128, 4, NQ], BF16)
    for qc in range(2):
        ps_st = cx.ps_half.tile([128, D], F32, tag="h")
        for dvt in range(4):
            nc.tensor.matmul(ps_st[0:1, :], lhsT=cx.onesc,
                             rhs=XT[:, dvt, 512 * qc:512 * (qc + 1)],
                             start=(dvt == 0), stop=(dvt == 3), tile_position=(0, 0))
            nc.tensor.matmul(ps_st[32:33, :], lhsT=cx.onesc,
                             rhs=SQ[:, dvt, 512 * qc:512 * (qc + 1)],
                             start=(dvt == 0), stop=(dvt == 3), tile_position=(0, 32))
        mu = cx.p_sml.tile([1, D], F32, tag="mu")
        nc.vector.tensor_scalar_mul(out=mu, in0=ps_st[0:1, :], scalar1=1.0 / D)
        mu2 = cx.p_sml.tile([1, D], F32, tag="mu2")
        nc.vector.tensor_tensor(out=mu2, in0=mu, in1=mu, op=OP.mult)
        var = cx.p_sml.tile([1, D], F32, tag="var")
        nc.vector.scalar_tensor_tensor(out=var, in0=ps_st[32:33, :],
                                       scalar=1.0 / D, in1=mu2,
                                       op0=OP.mult, op1=OP.subtract)
        lnv = cx.p_sml.tile([1, D], F32, tag="lnv")
        nc.scalar.activation(out=lnv, in_=var, func=AF.Ln, bias=cx.eps1, scale=1.0)
        rstd = cx.p_sml.tile([1, D], BF16, tag="rstd")
        nc.scalar.activation(out=rstd, in_=lnv, func=AF.Exp, scale=-0.5)
        nmr = cx.p_sml.tile([1, D], BF16, tag="nmr")
        nc.vector.scalar_tensor_tensor(out=nmr, in0=mu, scalar=-1.0, in1=rstd,
                                       op0=OP.mult, op1=OP.mult)
        ps_b2 = cx.ps_wide.tile([128, 2 * D], F32, tag="w")
        nc.tensor.matmul(ps_b2[:, 0:D], lhsT=cx.onesr, rhs=rstd, start=True, stop=True)
        nc.tensor.matmul(ps_b2[:, D:2 * D], lhsT=cx.onesr, rhs=nmr, start=True, stop=True)
        for dvt in range(4):
            t2 = cx.p_t1.tile([128, D], BF16, tag="t")
            nc.vector.tensor_tensor(out=t2, in0=XT[:, dvt, 512 * qc:512 * (qc + 1)],
                                    in1=ps_b2[:, 0:D], op=OP.mult)
            xn = XnT[:, dvt, 512 * qc:512 * (qc + 1)]
            nc.vector.tensor_tensor(out=xn, in0=t2, in1=ps_b2[:, D:2 * D], op=OP.add)
            if ln0_aff:
                nc.vector.tensor_scalar(out=xn, in0=xn,
                                        scalar1=cx.g04[:, dvt:dvt + 1],
                                        scalar2=cx.b04[:, dvt:dvt + 1],
                                        op0=OP.mult, op1=OP.add)
    return XnT


def _p5_out(nc, cx, XnT, dOut, rb, ln1_aff):
    xpre_l, mv_l = [], []
    vars8 = cx.p_sml.tile([128, 8], F32, tag="vars8")
    for nqt in range(8):
        ps_m = cx.ps_wide.tile([128, 2 * D], F32, tag="w")
        for dvt in range(4):
            lb = XnT[:, dvt, 128 * nqt:128 * (nqt + 1)]
            nc.tensor.matmul(ps_m[:, 0:D], lhsT=lb, rhs=cx.w_o[:, dvt, :],
                             start=(dvt == 0), stop=(dvt == 3))
            nc.tensor.matmul(ps_m[:, D:2 * D], lhsT=lb, rhs=cx.i512[:, dvt, :],
                             start=(dvt == 0), stop=(dvt == 3))
        rl = cx.p_t1.tile([128, D], BF16, tag="t")
        if cx.bob is not None:
            tb = cx.p_t1.tile([128, D], BF16, tag="t")
            nc.vector.tensor_tensor(out=tb, in0=cx.bob, in1=ps_m[:, 0:D], op=OP.add)
            nc.vector.tensor_scalar_max(out=rl, in0=tb, scalar1=0.0)
        else:
            nc.vector.tensor_scalar_max(out=rl, in0=ps_m[:, 0:D], scalar1=0.0)
        xpre = cx.p_xp.tile([128, D], F32)
        nc.vector.tensor_tensor(out=xpre, in0=rl, in1=ps_m[:, D:2 * D], op=OP.add)
        bst = cx.p_mv.tile([128, 6], F32, tag="bst")
        nc.vector.bn_stats(out=bst, in_=xpre)
        mv = cx.p_mv.tile([128, 2], F32, tag="mv")
        nc.vector.bn_aggr(out=mv, in_=bst)
        nc.vector.tensor_copy(out=vars8[:, nqt:nqt + 1], in_=mv[:, 1:2])
        xpre_l.append(xpre); mv_l.append(mv)
    lnv8 = cx.p_sml.tile([128, 8], F32, tag="lnv8")
    nc.scalar.activation(out=lnv8, in_=vars8, func=AF.Ln, bias=cx.epsP, scale=1.0)
    rstd8 = cx.p_sml.tile([128, 8], F32, tag="rstd8")
    nc.scalar.activation(out=rstd8, in_=lnv8, func=AF.Exp, scale=-0.5)
    for nqt in range(8):
        ot = cx.p_out.tile([128, D], F32)
        nc.vector.tensor_scalar(out=ot, in0=xpre_l[nqt],
                                scalar1=mv_l[nqt][:, 0:1],
                                scalar2=rstd8[:, nqt:nqt + 1],
                                op0=OP.subtract, op1=OP.mult)
        if ln1_aff:
            nc.vector.tensor_tensor(out=ot, in0=ot, in1=cx.g1b, op=OP.mult)
            nc.vector.tensor_tensor(out=ot, in0=ot, in1=cx.b1b, op=OP.add)
        nc.sync.dma_start(out=dOut[rb + 128 * nqt: rb + 128 * (nqt + 1), :], in_=ot)


def _build(flags, repeat=1):
    (bq_nz, bk_nz, bv_nz, bo_nz, ln0_aff, ln1_aff) = flags
    _pin_act_tables()
    nc = bacc.Bacc("TRN2", target_bir_lowering=False, debug=False,
                   num_devices=N_CORES)

    dQ = nc.dram_tensor("Qs", [BL * NQ, D], F32, kind="ExternalInput").ap()
    dK = nc.dram_tensor("Ks", [BL * NK, D], F32, kind="ExternalInput").ap()
    dOut = nc.dram_tensor("OUT", [BL * NQ, D], F32, kind="ExternalOutput").ap()

    cx = _Ctx()
    with ExitStack() as es:
        tc = es.enter_context(tile.TileContext(nc))
        ec = es.enter_context
        cst = ec(tc.tile_pool(name="cst", bufs=1))
        cx.p_qkt = ec(tc.tile_pool(name="qkt", bufs=1))
        cx.p_proj = ec(tc.tile_pool(name="proj", bufs=2))
        cx.p_xt = ec(tc.tile_pool(name="xt", bufs=1))
        cx.p_xnt = ec(tc.tile_pool(name="xnt", bufs=2))
        cx.p_nat = ec(tc.tile_pool(name="nat", bufs=2))
        cx.p_ex = ec(tc.tile_pool(name="ex", bufs=4))
        cx.p_ub = ec(tc.tile_pool(name="ub", bufs=2))
        cx.p_t1 = ec(tc.tile_pool(name="t1", bufs=3))
        cx.p_xp = ec(tc.tile_pool(name="xp", bufs=9))
        cx.p_out = ec(tc.tile_pool(name="outp", bufs=2))
        cx.p_sml = ec(tc.tile_pool(name="sml", bufs=2))
        cx.p_mv = ec(tc.tile_pool(name="mv", bufs=10))
        cx.ps_wide = ec(tc.tile_pool(name="wide", bufs=2, space="PSUM"))
        cx.ps_half = ec(tc.tile_pool(name="half", bufs=2, space="PSUM"))
        cx.ps_zz = ec(tc.tile_pool(name="zz", bufs=1, space="PSUM"))
        cx.ps_pv = ec(tc.tile_pool(name="pv", bufs=1, space="PSUM"))
        _setup_consts(nc, cx, cst, flags)

        def body():
            for b in range(BL):
                rb = b * NQ
                QT = _p1_transpose(nc, cx, rb, dQ, "QT")
                KT = _p1_transpose(nc, cx, rb, dK, "KT")
                qT, kT, vT = _p2_proj(nc, cx, QT, KT)
                XT = cx.p_xt.tile([128, 4, NQ], BF16, tag="XT")
                SQ = cx.p_xt.tile([128, 4, NQ], BF16, tag="SQ")
                for hp in range(4):
                    for qc in range(2):
                        _p3_attn_unit(nc, cx, qT, kT, vT, XT, SQ, hp, qc)
                XnT = _p4_ln0(nc, cx, XT, SQ, ln0_aff)
                _p5_out(nc, cx, XnT, dOut, rb, ln1_aff)

        if repeat == 1:
            body()
        else:
            with tc.For_i(0, repeat, 1):
                body()

    nc.compile()
    return nc


def _consts(Wq, Wk, Wv, Wo, flags, bq, bk, bv, bo, g0, b0, g1, b1):
    (bq_nz, bk_nz, bv_nz, bo_nz, ln0_aff, ln1_aff) = flags
    c = {
        "Wqb": np.ascontiguousarray(np.asarray(Wq).astype(NBF)),
        "Wkb": np.ascontiguousarray(np.asarray(Wk).astype(NBF)),
        "Wvb": np.ascontiguousarray(np.asarray(Wv).astype(NBF)),
        "Wob": np.ascontiguousarray(np.asarray(Wo).astype(NBF)),
        "I512b": np.eye(D, dtype=NBF),
        "I128f": np.eye(128, dtype=np.float32),
        "onesc": np.ones((128, 1), NBF),
        "onesr": np.ones((1, 128), NBF),
        "ones32": np.ones((128, 32), NBF),
    }
    mkJ = np.zeros((33, 128), NBF)
    mkJ[0, :64] = 1
    mkJ[32, 64:] = 1
    c["mkJ"] = mkJ
    if bq_nz: c["bq4"] = np.ascontiguousarray(np.asarray(bq).reshape(4, 128).T.astype(np.float32))
    if bk_nz: c["bk4"] = np.ascontiguousarray(np.asarray(bk).reshape(4, 128).T.astype(np.float32))
    if bv_nz: c["bvb"] = np.ascontiguousarray(np.broadcast_to(np.asarray(bv, np.float32), (128, D)))
    if bo_nz: c["bob"] = np.ascontiguousarray(np.broadcast_to(np.asarray(bo, np.float32), (128, D)))
    if ln0_aff:
        c["g04"] = np.ascontiguousarray(np.asarray(g0).reshape(4, 128).T.astype(np.float32))
        c["b04"] = np.ascontiguousarray(np.asarray(b0).reshape(4, 128).T.astype(np.float32))
    if ln1_aff:
        c["g1b"] = np.ascontiguousarray(np.broadcast_to(np.asarray(g1, np.float32), (128, D)))
        c["b1b"] = np.ascontiguousarray(np.broadcast_to(np.asarray(b1, np.float32), (128, D)))
    return c


def make_in_maps(Q, K, Wq, bq, Wk, bk, Wv, bv, Wo, bo, g0, b0, g1, b1, flags):
    consts = _consts(Wq, Wk, Wv, Wo, flags, bq, bk, bv, bo, g0, b0, g1, b1)
    in_maps = []
    for ci in range(N_CORES):
        m = dict(consts)
        m["Qs"] = np.ascontiguousarray(
            np.asarray(Q)[ci * BL:(ci + 1) * BL].reshape(BL * NQ, D).astype(np.float32))
        m["Ks"] = np.ascontiguousarray(
            np.asarray(K)[ci * BL:(ci + 1) * BL].reshape(BL * NK, D).astype(np.float32))
        in_maps.append(m)
    return in_maps


def get_flags(bq, bk, bv, bo, g0, b0, g1, b1):
    return (bool(np.any(np.asarray(bq))), bool(np.any(np.asarray(bk))),
            bool(np.any(np.asarray(bv))), bool(np.any(np.asarray(bo))),
            bool(np.any(np.asarray(g0) != 1) or np.any(np.asarray(b0))),
            bool(np.any(np.asarray(g1) != 1) or np.any(np.asarray(b1))))


def get_program(flags, repeat=1):
    key = (flags, repeat)
    if key not in _cache:
        _cache[key] = _build(flags, repeat)
    return _cache[key]


def kernel(Q, K, Wq, bq, Wk, bk, Wv, bv, Wo, bo, g0, b0, g1, b1):
    flags = get_flags(bq, bk, bv, bo, g0, b0, g1, b1)
    nc = get_program(flags, repeat=1)
    in_maps = make_in_maps(Q, K, Wq, bq, Wk, bk, Wv, bv, Wo, bo, g0, b0, g1, b1, flags)
    res = run_bass_kernel_spmd(nc, in_maps, list(range(N_CORES)))
    out = np.empty((B, NQ, D), np.float32)
    for ci in range(N_CORES):
        out[ci * BL:(ci + 1) * BL] = res.results[ci]["OUT"].reshape(BL, NQ, D)
    return out
